# revision 8
# baseline (speedup 1.0000x reference)
"""Trainium2 Bass kernel for nn_Attention_79645873537262.

Dense attention with per-head bias, key masking, sigmoid gate:
  t = x @ w_proj.T; per head: q,k,v
  a = softmax(scale*q@k.T + bias + mask); y = a@v
  y = sigmoid(x@w_g.T + b_g) * y;  out = y @ w_o.T + b_o

Sharding: tensor-parallel over heads, 2 heads per core on 8 cores.
Each core runs a fully independent program (no collectives): it computes
its 2 heads' attention plus its 128-column slice of the gate, and a
partial o_proj (contribution of its 128 y-columns to all 1024 outputs).
The host sums the 8 partial outputs and adds b_o (the "all-reduce").

On-device layout is transposed ("scores.T" flash style):
  scores.T[k,q] accumulated in PSUM as  ident@biasT (bias pre-masked,
  pre-transposed on host) + kT.T@qT ; exp on ScalarE (no max-subtraction:
  logits are ~N(0,2) by construction, |logit| < ~10 so exp is safe);
  y.T ext = [v | ones].T @ p gives y.T rows 0..63 and the softmax
  denominator in row 64. Normalization multiplies by a broadcast
  reciprocal (DMA DRAM round-trip to cross partitions).
All matmuls run in float32r (full-rate fp32, ~1.5e-4 relative rounding).
"""
import sys
import numpy as np

try:
    import concourse.bass as bass
except ImportError:
    sys.path.insert(0, "/opt/trn_rl_repo")
    import concourse.bass as bass

import concourse.tile as tile
from concourse import bacc, mybir
from concourse.bass_utils import run_bass_kernel_spmd

B, L, E, H = 1, 2048, 1024, 16
HW = E // H                # 64
SCALE = HW ** -0.5
N_CORES = 8
HPC = H // N_CORES         # 2 heads per core
C2 = HPC * HW              # 128 y-columns per core
MASK_NEG = -60.0           # exp(-60 + max_bias) ~ 1e-23: dead keys vanish

f32 = mybir.dt.float32
f32r = mybir.dt.float32r

NE = E // 128              # 8 contraction chunks
NQ = L // 512              # 4 q-tiles of 512
NKT = L // 128             # 16 k-chunks of 128

_compiled = [None]         # cache (nc) across kernel() calls
DEBUG = False


def _build():
    nc = bacc.Bacc("TRN2", target_bir_lowering=False, debug=False,
                   num_devices=N_CORES)

    xT_ap = nc.dram_tensor("xT", [E, L], f32r, kind="ExternalInput").ap()
    wpT_ap = nc.dram_tensor("wpT", [E, 3 * C2], f32r, kind="ExternalInput").ap()
    biasT_ap = nc.dram_tensor("biasT", [HPC, L, L], f32r, kind="ExternalInput").ap()
    wgT_ap = nc.dram_tensor("wgT", [E, C2], f32r, kind="ExternalInput").ap()
    bgv_ap = nc.dram_tensor("bgv", [C2, 1], f32, kind="ExternalInput").ap()
    woT_ap = nc.dram_tensor("woT", [C2, E], f32r, kind="ExternalInput").ap()
    ident_ap = nc.dram_tensor("ident", [128, 128], f32r, kind="ExternalInput").ap()
    onescol_ap = nc.dram_tensor("onescol", [128, 1], f32r, kind="ExternalInput").ap()
    outT_ap = nc.dram_tensor("outT", [E, L], f32, kind="ExternalOutput").ap()
    if DEBUG:
        dbg_q01_ap = nc.dram_tensor("dbg_q01", [128, L], f32, kind="ExternalOutput").ap()
        dbg_k01_ap = nc.dram_tensor("dbg_k01", [128, L], f32, kind="ExternalOutput").ap()
        dbg_v0_ap = nc.dram_tensor("dbg_v0", [128, 130], f32, kind="ExternalOutput").ap()
        dbg_g_ap = nc.dram_tensor("dbg_g", [128, L], f32, kind="ExternalOutput").ap()
        dbg_ygT_ap = nc.dram_tensor("dbg_ygT", [128, L], f32, kind="ExternalOutput").ap()
        dbg_sums_ap = nc.dram_tensor("dbg_sums", [HPC, L], f32, kind="ExternalOutput").ap()
        dbg_rb_ap = nc.dram_tensor("dbg_rb", [HPC, 64, L], f32, kind="ExternalOutput").ap()
        dbg_p_ap = nc.dram_tensor("dbg_p", [128, 1024], f32, kind="ExternalOutput").ap()
        dbg_rcp_ap = nc.dram_tensor("dbg_rcp", [HPC, L], f32, kind="ExternalOutput").ap()

    with tile.TileContext(nc) as tc:
        from contextlib import ExitStack
        with ExitStack() as persist:
            pers = persist.enter_context(tc.tile_pool(name="pers", bufs=1))
            # persistent SBUF tensors
            q01 = pers.tile([128, L], f32r, tag="q01")   # rows 0:64 q_h0*scale, 64:128 q_h1*scale
            k01 = pers.tile([128, L], f32r, tag="k01")
            v_sb = [pers.tile([128, 130], f32r, name=f"v{kt}", tag=f"v{kt}") for kt in range(NKT)]
            g_sb = pers.tile([128, L], f32r, tag="g")
            ygT = pers.tile([128, L], f32r, tag="ygT")
            woT_sb = pers.tile([C2, E], f32r, tag="woT")
            ident_sb = pers.tile([128, 128], f32r, tag="ident")
            nc.sync.dma_start(woT_sb, woT_ap)
            nc.sync.dma_start(ident_sb, ident_ap)
            # ones columns of v_ext (col 64 for head0, col 129 for head1)
            for kt in range(NKT):
                nc.sync.dma_start(v_sb[kt][:, 64:65], onescol_ap)
                nc.sync.dma_start(v_sb[kt][:, 129:130], onescol_ap)

            # ---------------- phase 1: proj + gate ----------------
            with tc.tile_pool(name="ph1", bufs=1) as ph1, \
                 tc.tile_pool(name="pj", bufs=2, space="PSUM") as pj, \
                 tc.tile_pool(name="tr", bufs=2, space="PSUM") as trp, \
                 tc.tile_pool(name="pg", bufs=2, space="PSUM") as pg:
                xT_sb = []
                for e in range(NE):
                    t = ph1.tile([128, L], f32r, tag=f"xT{e}")
                    nc.sync.dma_start(t, xT_ap[e * 128:(e + 1) * 128, :])
                    xT_sb.append(t)
                wpT_sb = ph1.tile([128, NE, 3 * C2], f32r, tag="wpT")
                nc.sync.dma_start(wpT_sb, wpT_ap.rearrange("(c p) m -> p c m", p=128))
                wgT_sb = ph1.tile([128, NE, C2], f32r, tag="wgT")
                nc.sync.dma_start(wgT_sb, wgT_ap.rearrange("(c p) m -> p c m", p=128))
                bgv_sb = ph1.tile([C2, 1], f32, tag="bgv")
                nc.sync.dma_start(bgv_sb, bgv_ap)
                vT01 = ph1.tile([128, L], f32r, tag="vT01")

                # proj: chunks f=0 (q01), 1 (k01), 2 (vT01)
                dests = [q01, k01, vT01]
                for f in range(3):
                    for lt in range(NQ):
                        ps = pj.tile([128, 512], f32, tag="pj")
                        for e in range(NE):
                            nc.tensor.matmul(
                                ps,
                                wpT_sb[:, e, f * 128:(f + 1) * 128],
                                xT_sb[e][:, lt * 512:(lt + 1) * 512],
                                start=(e == 0), stop=(e == NE - 1))
                        nc.vector.tensor_copy(
                            dests[f][:, lt * 512:(lt + 1) * 512], ps)

                # transpose vT01 -> v_sb tiles [l, c2]
                for kt in range(NKT):
                    ps = trp.tile([128, 128], f32r, tag="tr")
                    nc.tensor.transpose(
                        ps, vT01[:, kt * 128:(kt + 1) * 128], ident_sb)
                    nc.vector.tensor_copy(v_sb[kt][:, 0:64], ps[:, 0:64])
                    nc.vector.tensor_copy(v_sb[kt][:, 65:129], ps[:, 64:128])

                # gate: g = sigmoid(wgT.T @ xT + bg)
                for lt in range(NQ):
                    ps = pg.tile([C2, 512], f32, tag="pg")
                    for e in range(NE):
                        nc.tensor.matmul(
                            ps, wgT_sb[:, e, :],
                            xT_sb[e][:, lt * 512:(lt + 1) * 512],
                            start=(e == 0), stop=(e == NE - 1))
                    nc.scalar.activation(
                        g_sb[:, lt * 512:(lt + 1) * 512], ps,
                        mybir.ActivationFunctionType.Sigmoid,
                        bias=bgv_sb, scale=1.0)

            # ---------------- phase 2: attention ----------------
            with tc.tile_pool(name="bias", bufs=3) as biasp, \
                 tc.tile_pool(name="pp", bufs=3) as pp, \
                 tc.tile_pool(name="nrm", bufs=2) as nrm, \
                 tc.tile_pool(name="dram", bufs=2, space="DRAM") as dramp, \
                 tc.tile_pool(name="s", bufs=2, space="PSUM") as sp, \
                 tc.tile_pool(name="y", bufs=1, space="PSUM") as yp:
                for h in range(HPC):
                    hb = h * 64
                    y_ps = [yp.tile([65, 512], f32, name=f"y{h}_{i}", tag=f"yq{i}") for i in range(NQ)]
                    for kt in range(NKT):
                        bias_t = biasp.tile([128, L], f32r, tag="bias")
                        nc.sync.dma_start(
                            bias_t, biasT_ap[h, kt * 128:(kt + 1) * 128, :])
                        for half in range(2):
                            s_ps = sp.tile([128, 1024], f32, tag="s")
                            for qq in range(2):
                                qs = half * 1024 + qq * 512
                                nc.tensor.matmul(
                                    s_ps[:, qq * 512:(qq + 1) * 512],
                                    ident_sb, bias_t[:, qs:qs + 512],
                                    start=True, stop=False)
                                nc.tensor.matmul(
                                    s_ps[:, qq * 512:(qq + 1) * 512],
                                    k01[hb:hb + 64, kt * 128:(kt + 1) * 128],
                                    q01[hb:hb + 64, qs:qs + 512],
                                    start=False, stop=True)
                            p_t = pp.tile([128, 1024], f32r, tag="p")
                            nc.scalar.activation(
                                p_t, s_ps, mybir.ActivationFunctionType.Exp)
                            if DEBUG and h == 0 and kt == 0 and half == 0:
                                nc.sync.dma_start(dbg_p_ap, p_t.bitcast(f32))
                            for qq in range(2):
                                nc.tensor.matmul(
                                    y_ps[half * 2 + qq],
                                    v_sb[kt][:, h * 65:(h + 1) * 65],
                                    p_t[:, qq * 512:(qq + 1) * 512],
                                    start=(kt == 0), stop=(kt == NKT - 1))
                    # normalization: sums live in row 64 of y_ps tiles
                    sums_sb = nrm.tile([65, L], f32, tag="sums")
                    for qt in range(NQ):
                        nc.vector.tensor_copy(
                            sums_sb[64:65, qt * 512:(qt + 1) * 512],
                            y_ps[qt][64:65, :])
                    if DEBUG:
                        nc.sync.dma_start(dbg_sums_ap[h:h+1, :], sums_sb[64:65, :])
                    dscr = dramp.tile([1, L], f32, tag="dscr")
                    nc.sync.dma_start(dscr, sums_sb[64:65, :])
                    sums_b = nrm.tile([64, L], f32, tag="sums_b")
                    nc.sync.dma_start(sums_b, dscr.partition_broadcast(64))
                    rb_sb = nrm.tile([64, L], f32, tag="rb")
                    nc.vector.reciprocal_approx_fast(rb_sb, sums_b)
                    if DEBUG:
                        nc.sync.dma_start(dbg_rcp_ap[h:h+1, :], rb_sb[0:1, :])
                    if DEBUG:
                        nc.sync.dma_start(dbg_rb_ap[h], rb_sb)
                    if h == 0:
                        for qt in range(NQ):
                            nc.vector.tensor_mul(
                                ygT[0:64, qt * 512:(qt + 1) * 512],
                                y_ps[qt][0:64, :],
                                rb_sb[:, qt * 512:(qt + 1) * 512])
                    else:
                        yg1 = nrm.tile([64, L], f32r, tag="yg1")
                        for qt in range(NQ):
                            nc.vector.tensor_mul(
                                yg1[:, qt * 512:(qt + 1) * 512],
                                y_ps[qt][0:64, :],
                                rb_sb[:, qt * 512:(qt + 1) * 512])
                        nc.sync.dma_start(ygT[64:128, :], yg1)

            # gate multiply (in place)
            nc.vector.tensor_mul(ygT, ygT, g_sb)
            if DEBUG:
                nc.sync.dma_start(dbg_q01_ap, q01.bitcast(f32))
                nc.sync.dma_start(dbg_k01_ap, k01.bitcast(f32))
                nc.sync.dma_start(dbg_v0_ap, v_sb[0].bitcast(f32))
                nc.sync.dma_start(dbg_g_ap, g_sb.bitcast(f32))
                nc.sync.dma_start(dbg_ygT_ap, ygT.bitcast(f32))

            # ---------------- phase 3: o_proj partial ----------------
            with tc.tile_pool(name="outp", bufs=2) as outp, \
                 tc.tile_pool(name="po", bufs=4, space="PSUM") as pop:
                for eo in range(NE):
                    ot = outp.tile([128, L], f32, tag="ot")
                    for qt in range(NQ):
                        ps = pop.tile([128, 512], f32, tag="po")
                        nc.tensor.matmul(
                            ps, woT_sb[:, eo * 128:(eo + 1) * 128],
                            ygT[:, qt * 512:(qt + 1) * 512],
                            start=True, stop=True)
                        nc.vector.tensor_copy(
                            ot[:, qt * 512:(qt + 1) * 512], ps)
                    nc.sync.dma_start(
                        outT_ap[eo * 128:(eo + 1) * 128, :], ot)

    nc.compile()
    return nc


def kernel(x, mask, bias, w_proj, w_o, b_o, w_g, b_g):
    x = np.asarray(x, dtype=np.float32)
    mask = np.asarray(mask)
    bias = np.asarray(bias, dtype=np.float32)
    w_proj = np.asarray(w_proj, dtype=np.float32)
    w_o = np.asarray(w_o, dtype=np.float32)
    b_o = np.asarray(b_o, dtype=np.float32)
    w_g = np.asarray(w_g, dtype=np.float32)
    b_g = np.asarray(b_g, dtype=np.float32)

    if _compiled[0] is None:
        _compiled[0] = _build()
    nc = _compiled[0]

    xT = np.ascontiguousarray(x[0].T)                      # [E, L]
    mask_add = np.where(mask[0], 0.0, MASK_NEG).astype(np.float32)  # [L]
    ident = np.eye(128, dtype=np.float32)
    onescol = np.ones((128, 1), dtype=np.float32)

    in_maps = []
    for c in range(N_CORES):
        heads = [c * HPC + i for i in range(HPC)]
        # w_proj rows for head h: [h*3HW, (h+1)*3HW) = [q | k | v] each HW rows
        wpT = np.empty((E, 3 * C2), dtype=np.float32)
        for i, h in enumerate(heads):
            r0 = h * 3 * HW
            wpT[:, 0 * C2 + i * HW: 0 * C2 + (i + 1) * HW] = \
                w_proj[r0: r0 + HW].T * SCALE               # q, pre-scaled
            wpT[:, 1 * C2 + i * HW: 1 * C2 + (i + 1) * HW] = \
                w_proj[r0 + HW: r0 + 2 * HW].T              # k
            wpT[:, 2 * C2 + i * HW: 2 * C2 + (i + 1) * HW] = \
                w_proj[r0 + 2 * HW: r0 + 3 * HW].T          # v
        biasT = np.ascontiguousarray(
            bias[0, :, :, heads].transpose(0, 2, 1))        # [2, Lk, Lq]
        biasT += mask_add[None, :, None]
        cols = slice(c * C2, (c + 1) * C2)
        wgT = np.ascontiguousarray(w_g[cols, :].T)          # [E, C2]
        bgv = np.ascontiguousarray(b_g[cols, None])         # [C2, 1]
        woT = np.ascontiguousarray(w_o[:, cols].T)          # [C2, E]
        in_maps.append({
            "xT": xT, "wpT": wpT, "biasT": biasT, "wgT": wgT,
            "bgv": bgv, "woT": woT, "ident": ident, "onescol": onescol,
        })

    res = run_bass_kernel_spmd(nc, in_maps, list(range(N_CORES)))
    acc = res.results[0]["outT"].astype(np.float64)
    for c in range(1, N_CORES):
        acc += res.results[c]["outT"]
    out = acc.T.astype(np.float32) + b_o[None, :]
    return out[None]  # [B, L, E]


# revision 12
# speedup vs baseline: 1.1539x; 1.1539x over previous
"""Trainium2 Bass kernel for nn_Attention_79645873537262.

Dense attention with per-head bias, key masking, sigmoid gate:
  t = x @ w_proj.T; per head: q,k,v
  a = softmax(scale*q@k.T + bias + mask); y = a@v
  y = sigmoid(x@w_g.T + b_g) * y;  out = y @ w_o.T + b_o

Sharding: tensor-parallel over heads, 2 heads per core on 8 cores.
Each core runs a fully independent program (no collectives): it computes
its 2 heads' attention plus its 128-column slice of the gate, and a
partial o_proj (contribution of its 128 y-columns to all 1024 outputs).
The host sums the 8 partial outputs and adds b_o (the "all-reduce").

On-device layout is transposed ("scores.T" flash style):
  scores.T[k,q] accumulated in PSUM as  ident@biasT (bias pre-masked,
  pre-transposed on host) + kT.T@qT ; exp on ScalarE (no max-subtraction:
  logits are ~N(0,2) by construction, |logit| < ~14 so exp is safe);
  y.T ext = [v | ones].T @ p gives y.T rows 0..63 and the softmax
  denominator in row 64. Normalization multiplies by a broadcast
  reciprocal (DMA DRAM round-trip to cross partitions).
All matmuls run in float32r (full-rate fp32, ~1.5e-4 relative rounding).

Perf notes (from NTFF traces): every f32r matmul pays a serialized
~LDWEIGHTS+MATMUL pair (~426 ns warm); HAM re-warm never triggers in
this instruction mix, so the kernel must never let the PE idle >3.4us:
one PSUM pool layout for all phases (no pool-transition barrier),
proj-critical DMAs dispatched first, bias stream on the (otherwise
idle) GpSimd DGE queue, and a per-q-tile tail so o_proj/output DMA
overlap the end of attention.
"""
import sys
import numpy as np

try:
    import concourse.bass as bass
except ImportError:
    sys.path.insert(0, "/opt/trn_rl_repo")
    import concourse.bass as bass

import concourse.tile as tile
from concourse import bacc, mybir
from concourse.bass_utils import run_bass_kernel_spmd

B, L, E, H = 1, 2048, 1024, 16
HW = E // H                # 64
SCALE = HW ** -0.5
N_CORES = 8
HPC = H // N_CORES         # 2 heads per core
C2 = HPC * HW              # 128 y-columns per core
MASK_NEG = -60.0           # exp(-60 + max_bias) ~ 1e-23: dead keys vanish

f32 = mybir.dt.float32
f32r = mybir.dt.float32r

NE = E // 128              # 8 contraction chunks
NQ = L // 512              # 4 q-tiles of 512
NKT = L // 128             # 16 k-chunks of 128

_compiled = [None]
DEBUG = False


def _build():
    nc = bacc.Bacc("TRN2", target_bir_lowering=False, debug=False,
                   num_devices=N_CORES)

    xT_ap = nc.dram_tensor("xT", [E, L], f32r, kind="ExternalInput").ap()
    wpT_ap = nc.dram_tensor("wpT", [E, 3 * C2], f32r, kind="ExternalInput").ap()
    biasT_ap = nc.dram_tensor("biasT", [HPC, L, L], f32r, kind="ExternalInput").ap()
    wgT_ap = nc.dram_tensor("wgT", [E, C2], f32r, kind="ExternalInput").ap()
    bgv_ap = nc.dram_tensor("bgv", [C2, 1], f32, kind="ExternalInput").ap()
    woT_ap = nc.dram_tensor("woT", [C2, E], f32r, kind="ExternalInput").ap()
    ident_ap = nc.dram_tensor("ident", [128, 128], f32r, kind="ExternalInput").ap()
    onescols_ap = nc.dram_tensor("onescols", [128, NKT], f32r, kind="ExternalInput").ap()
    outT_ap = nc.dram_tensor("outT", [E, L], f32, kind="ExternalOutput").ap()

    with tile.TileContext(nc) as tc:
        from contextlib import ExitStack
        with ExitStack() as ctx:
            pers = ctx.enter_context(tc.tile_pool(name="pers", bufs=1))
            work = ctx.enter_context(tc.tile_pool(name="work", bufs=1))
            biasp = ctx.enter_context(tc.tile_pool(name="bias", bufs=3))
            pp = ctx.enter_context(tc.tile_pool(name="pp", bufs=3))
            nrm = ctx.enter_context(tc.tile_pool(name="nrm", bufs=1))
            dramp = ctx.enter_context(tc.tile_pool(name="dram", bufs=2, space="DRAM"))
            outp = ctx.enter_context(tc.tile_pool(name="outp", bufs=3))
            # one PSUM layout for the whole kernel: no pool-transition barrier
            sp = ctx.enter_context(tc.tile_pool(name="s", bufs=2, space="PSUM"))
            yp = ctx.enter_context(tc.tile_pool(name="y", bufs=1, space="PSUM"))

            # --- proj-critical DMAs first (dispatch order matters) ---
            xT_sb = []
            for e in range(NE):
                t = pers.tile([128, L], f32r, name=f"xT{e}", tag=f"xT{e}")
                nc.sync.dma_start(t, xT_ap[e * 128:(e + 1) * 128, :])
                xT_sb.append(t)
            wpT_sb = pers.tile([128, NE, 3 * C2], f32r, tag="wpT")
            nc.sync.dma_start(wpT_sb, wpT_ap.rearrange("(c p) m -> p c m", p=128))
            ident_sb = pers.tile([128, 128], f32r, tag="ident")
            nc.sync.dma_start(ident_sb, ident_ap)
            wgT_sb = pers.tile([128, NE, C2], f32r, tag="wgT")
            nc.sync.dma_start(wgT_sb, wgT_ap.rearrange("(c p) m -> p c m", p=128))
            bgv_sb = pers.tile([C2, 1], f32, tag="bgv")
            nc.sync.dma_start(bgv_sb, bgv_ap)
            woT_sb = pers.tile([C2, E], f32r, tag="woT")
            nc.sync.dma_start(woT_sb, woT_ap)
            # v tiles: [128 l, 130] per k-chunk: [v_h0 | ones | v_h1 | ones]
            v_all = pers.tile([128, NKT, 130], f32r, tag="v_all")
            nc.sync.dma_start(v_all[:, :, 64:65], onescols_ap.unsqueeze(2))
            nc.sync.dma_start(v_all[:, :, 129:130], onescols_ap.unsqueeze(2))

            q01 = pers.tile([128, L], f32r, tag="q01")
            k01 = pers.tile([128, L], f32r, tag="k01")
            g_sb = pers.tile([128, L], f32r, tag="g")
            ygT = pers.tile([128, L], f32r, tag="ygT")

            # --- bias stream on GpSimd DGE (keeps Sync queue clear) ---
            bias_tiles = []
            for h in range(HPC):
                for kt in range(NKT):
                    bt = biasp.tile([128, L], f32r, name=f"bias{h}_{kt}", tag="bias")
                    nc.gpsimd.dma_start(
                        bt, biasT_ap[h, kt * 128:(kt + 1) * 128, :])
                    bias_tiles.append(bt)

            # ---------------- proj ----------------
            vT01 = work.tile([128, L], f32r, tag="vT01")
            dests = [q01, k01, vT01]
            for f in range(3):
                for lt in range(NQ):
                    ps = sp.tile([128, 1024], f32, name=f"pj{f}_{lt}", tag="s")
                    for e in range(NE):
                        nc.tensor.matmul(
                            ps[:, 0:512],
                            wpT_sb[:, e, f * 128:(f + 1) * 128],
                            xT_sb[e][:, lt * 512:(lt + 1) * 512],
                            start=(e == 0), stop=(e == NE - 1))
                    nc.vector.tensor_copy(
                        dests[f][:, lt * 512:(lt + 1) * 512], ps[:, 0:512])

            # transpose vT01 -> v_all[:, kt, :]
            for kt in range(NKT):
                ps = sp.tile([128, 128], f32r, name=f"tr{kt}", tag="s")
                nc.tensor.transpose(
                    ps, vT01[:, kt * 128:(kt + 1) * 128], ident_sb)
                nc.vector.tensor_copy(v_all[:, kt, 0:64], ps[:, 0:64])
                nc.vector.tensor_copy(v_all[:, kt, 65:129], ps[:, 64:128])

            # gate: g = sigmoid(wgT.T @ xT + bg)
            for lt in range(NQ):
                ps = sp.tile([C2, 1024], f32, name=f"pg{lt}", tag="s")
                for e in range(NE):
                    nc.tensor.matmul(
                        ps[:, 0:512], wgT_sb[:, e, :],
                        xT_sb[e][:, lt * 512:(lt + 1) * 512],
                        start=(e == 0), stop=(e == NE - 1))
                nc.scalar.activation(
                    g_sb[:, lt * 512:(lt + 1) * 512], ps[:, 0:512],
                    mybir.ActivationFunctionType.Sigmoid,
                    bias=bgv_sb, scale=1.0)

            if DEBUG:
                dbg_q01_ap = nc.dram_tensor("dbg_q01", [128, L], f32, kind="ExternalOutput").ap()
                dbg_sums_ap = nc.dram_tensor("dbg_sums", [HPC, L], f32, kind="ExternalOutput").ap()
                nc.sync.dma_start(dbg_q01_ap, q01.bitcast(f32))

            # ---------------- attention ----------------
            for h in range(HPC):
                hb = h * 64
                y_ps = [yp.tile([65, 512], f32, name=f"y{h}_{i}", tag=f"yq{i}")
                        for i in range(NQ)]
                for kt in range(NKT):
                    bias_t = bias_tiles[h * NKT + kt]
                    for half in range(2):
                        s_ps = sp.tile([128, 1024], f32,
                                       name=f"s{h}_{kt}_{half}", tag="s")
                        for qq in range(2):
                            qs = half * 1024 + qq * 512
                            nc.tensor.matmul(
                                s_ps[:, qq * 512:(qq + 1) * 512],
                                ident_sb, bias_t[:, qs:qs + 512],
                                start=True, stop=False)
                            nc.tensor.matmul(
                                s_ps[:, qq * 512:(qq + 1) * 512],
                                k01[hb:hb + 64, kt * 128:(kt + 1) * 128],
                                q01[hb:hb + 64, qs:qs + 512],
                                start=False, stop=True)
                        p_t = pp.tile([128, 1024], f32r,
                                      name=f"p{h}_{kt}_{half}", tag="p")
                        nc.scalar.activation(
                            p_t, s_ps, mybir.ActivationFunctionType.Exp)
                        for qq in range(2):
                            nc.tensor.matmul(
                                y_ps[half * 2 + qq],
                                v_all[:, kt, h * 65:(h + 1) * 65],
                                p_t[:, qq * 512:(qq + 1) * 512],
                                start=(kt == 0), stop=(kt == NKT - 1))
                # normalization: softmax denominators are row 64 of y_ps
                sums_sb = nrm.tile([65, L], f32, name=f"sums{h}", tag="sums")
                for qt in range(NQ):
                    nc.vector.tensor_copy(
                        sums_sb[64:65, qt * 512:(qt + 1) * 512],
                        y_ps[qt][64:65, :])
                if DEBUG:
                    nc.sync.dma_start(dbg_sums_ap[h:h+1, :], sums_sb[64:65, :])
                dscr = dramp.tile([1, L], f32, name=f"dscr{h}", tag="dscr")
                nc.sync.dma_start(dscr, sums_sb[64:65, :])
                sums_b = nrm.tile([64, L], f32, name=f"sums_b{h}", tag="sums_b")
                nc.sync.dma_start(sums_b, dscr.partition_broadcast(64))
                rb_sb = nrm.tile([64, L], f32, name=f"rb{h}", tag="rb")
                nc.vector.reciprocal_approx_fast(rb_sb, sums_b)
                if h == 0:
                    for qt in range(NQ):
                        nc.vector.tensor_mul(
                            ygT[0:64, qt * 512:(qt + 1) * 512],
                            y_ps[qt][0:64, :],
                            rb_sb[:, qt * 512:(qt + 1) * 512])
                else:
                    yg1 = nrm.tile([64, L], f32r, name="yg1", tag="yg1")
                    for qt in range(NQ):
                        nc.vector.tensor_mul(
                            yg1[:, qt * 512:(qt + 1) * 512],
                            y_ps[qt][0:64, :],
                            rb_sb[:, qt * 512:(qt + 1) * 512])
                        nc.sync.dma_start(
                            ygT[64:128, qt * 512:(qt + 1) * 512],
                            yg1[:, qt * 512:(qt + 1) * 512])

            # ---------------- tail: gate mul + o_proj, per q-tile ----------------
            for qt in range(NQ):
                qsl = slice(qt * 512, (qt + 1) * 512)
                nc.vector.tensor_mul(ygT[:, qsl], ygT[:, qsl], g_sb[:, qsl])
                for eo in range(NE):
                    ps = sp.tile([128, 1024], f32, name=f"po{qt}_{eo}", tag="s")
                    nc.tensor.matmul(
                        ps[:, 0:512], woT_sb[:, eo * 128:(eo + 1) * 128],
                        ygT[:, qsl], start=True, stop=True)
                    ot = outp.tile([128, 512], f32, name=f"ot{qt}_{eo}", tag="ot")
                    nc.vector.tensor_copy(ot, ps[:, 0:512])
                    nc.sync.dma_start(
                        outT_ap[eo * 128:(eo + 1) * 128, qsl], ot)

    nc.compile()
    return nc


def kernel(x, mask, bias, w_proj, w_o, b_o, w_g, b_g):
    x = np.asarray(x, dtype=np.float32)
    mask = np.asarray(mask)
    bias = np.asarray(bias, dtype=np.float32)
    w_proj = np.asarray(w_proj, dtype=np.float32)
    w_o = np.asarray(w_o, dtype=np.float32)
    b_o = np.asarray(b_o, dtype=np.float32)
    w_g = np.asarray(w_g, dtype=np.float32)
    b_g = np.asarray(b_g, dtype=np.float32)

    if _compiled[0] is None:
        _compiled[0] = _build()
    nc = _compiled[0]

    xT = np.ascontiguousarray(x[0].T)                      # [E, L]
    mask_add = np.where(mask[0], 0.0, MASK_NEG).astype(np.float32)  # [L]
    ident = np.eye(128, dtype=np.float32)
    onescols = np.ones((128, NKT), dtype=np.float32)

    in_maps = []
    for c in range(N_CORES):
        heads = [c * HPC + i for i in range(HPC)]
        wpT = np.empty((E, 3 * C2), dtype=np.float32)
        for i, h in enumerate(heads):
            r0 = h * 3 * HW
            wpT[:, 0 * C2 + i * HW: 0 * C2 + (i + 1) * HW] = \
                w_proj[r0: r0 + HW].T * SCALE               # q, pre-scaled
            wpT[:, 1 * C2 + i * HW: 1 * C2 + (i + 1) * HW] = \
                w_proj[r0 + HW: r0 + 2 * HW].T              # k
            wpT[:, 2 * C2 + i * HW: 2 * C2 + (i + 1) * HW] = \
                w_proj[r0 + 2 * HW: r0 + 3 * HW].T          # v
        biasT = np.ascontiguousarray(
            bias[0, :, :, heads].transpose(0, 2, 1))        # [2, Lk, Lq]
        biasT += mask_add[None, :, None]
        cols = slice(c * C2, (c + 1) * C2)
        wgT = np.ascontiguousarray(w_g[cols, :].T)          # [E, C2]
        bgv = np.ascontiguousarray(b_g[cols, None])         # [C2, 1]
        woT = np.ascontiguousarray(w_o[:, cols].T)          # [C2, E]
        in_maps.append({
            "xT": xT, "wpT": wpT, "biasT": biasT, "wgT": wgT,
            "bgv": bgv, "woT": woT, "ident": ident, "onescols": onescols,
        })

    res = run_bass_kernel_spmd(nc, in_maps, list(range(N_CORES)))
    acc = res.results[0]["outT"].astype(np.float64)
    for c in range(1, N_CORES):
        acc += res.results[c]["outT"]
    out = acc.T.astype(np.float32) + b_o[None, :]
    return out[None]  # [B, L, E]
